# revision 18
# baseline (speedup 1.0000x reference)
"""LogNorm moment-matching kernel for Trainium2, sharded over 8 NeuronCores.

Math: given Z_mu, Z_sigma (C=64, G=20000), corr (C,C), cell_prob (S=16, C):
  ev    = exp(Z_mu + 0.5*Z_sigma^2)
  Ssum  = cell_prob @ ev                                  (S, G)
  var_n[s,g] = sum_{c,d} A[s,c,g] corr[c,d] A[s,d,g],  A = cell_prob[s,c]*F[c,g],
               F = ev * Z_sigma
With corr = R R^T (Cholesky):  var_n[s] = || R^T diag(cp[s]) F ||^2
  -> per state: one 64x64 matmul (2 states packed per 128-col lhsT), squares of
     the result, then a partition-sum done as an accumulating matmul with a 0/1
     selector lhsT.
  var = var_n / Ssum^2; mu = ln(Ssum) - var/2; sigma = sqrt(var)
  done via the exp/ln table set only: c = ln(var_n) - 2 ln(Ssum),
  sigma = exp(0.5 c), var = exp(c).

Sharding: genes split 20000 -> 8 x 2500; corr/cell_prob-derived constants
replicated. All G-sized compute on device.

Engine split per gene chunk (NT=512): PE: 8 Y-matmuls + 8 reduce + 1 Ssum.
ACT: square-prep, exp, 2 of 4 double-pair squares (PSUM->SBUF). DVE: other 2
doubles (after DMA PSUM->SBUF evict). GPSIMD: the two SBUF elementwise muls.
Finals on a (80, 512) restacked layout to use full partitions.
"""

import sys

sys.path.insert(0, "/opt/trn_rl_repo")

import numpy as np

import concourse.bass as bass
import concourse.tile as tile
from concourse import bacc, mybir
from concourse.bass_utils import run_bass_kernel_spmd

S, C, G = 16, 64, 20000
NCORES = 8
GLOC = G // NCORES          # 2500 genes per core
NT = 512                    # gene tile = one PSUM bank of f32
NPAIR = S // 2
NDBL = NPAIR // 2           # 4 double-pair PSUM tiles per chunk
ACT_DOUBLES = 3             # doubles squared directly by ACT; rest DVE copy+mul

_cache = {}


def _chunks():
    out = []
    g = 0
    while g < GLOC:
        n = min(NT, GLOC - g)
        out.append((g, n))
        g += n
    return out


CHUNKS = _chunks()
NCHUNK = len(CHUNKS)


def _build():
    nc = bacc.Bacc("TRN2", target_bir_lowering=False)
    dt = mybir.dt.float32

    zmu_d = nc.dram_tensor("zmu", [C, GLOC], dt, kind="ExternalInput")
    zs_d = nc.dram_tensor("zs", [C, GLOC], dt, kind="ExternalInput")
    lhsp_d = nc.dram_tensor("lhsp", [C, NPAIR, 128], dt, kind="ExternalInput")
    cpt_d = nc.dram_tensor("cpt", [C, S], dt, kind="ExternalInput")
    ones_d = nc.dram_tensor("ones", [128, NPAIR, S], dt, kind="ExternalInput")
    mu_d = nc.dram_tensor("mu", [S, GLOC], dt, kind="ExternalOutput")
    sg_d = nc.dram_tensor("sigma", [S, GLOC], dt, kind="ExternalOutput")

    HALF_SQ = float(np.sqrt(0.5))
    SP = S * NCHUNK  # 80 partitions for the restacked tail

    with tile.TileContext(nc) as tc:
        with (
            tc.tile_pool(name="consts", bufs=1) as consts,
            tc.tile_pool(name="inp", bufs=3) as inp,
            tc.tile_pool(name="wk", bufs=3) as wk,
            tc.tile_pool(name="xp", bufs=4) as xp,
            tc.tile_pool(name="ye", bufs=3) as ye,
            tc.tile_pool(name="big", bufs=1) as big,
            tc.tile_pool(name="ps_y", bufs=3, space="PSUM") as ps_y,
            tc.tile_pool(name="ps_s", bufs=1, space="PSUM") as ps_s,
            tc.tile_pool(name="ps_v", bufs=1, space="PSUM") as ps_v,
        ):
            lhsp_t = consts.tile([C, NPAIR, 128], dt)
            nc.sync.dma_start(out=lhsp_t, in_=lhsp_d[:, :, :])
            cpt_t = consts.tile([C, S], dt)
            nc.sync.dma_start(out=cpt_t, in_=cpt_d[:, :])
            ones_t = consts.tile([128, NPAIR, S], dt)
            nc.sync.dma_start(out=ones_t, in_=ones_d[:, :, :])

            bigS = big.tile([SP, NT], dt)
            bigV = big.tile([SP, NT], dt)

            for i, (g0, n) in enumerate(CHUNKS):
                sl = slice(g0, g0 + n)
                rows = slice(S * i, S * (i + 1))
                zmu_t = inp.tile([C, NT], dt, tag="zmu")
                nc.gpsimd.dma_start(out=zmu_t[:, :n], in_=zmu_d[:, sl])
                zs_t = inp.tile([C, NT], dt, tag="zs")
                nc.gpsimd.dma_start(out=zs_t[:, :n], in_=zs_d[:, sl])

                # ev = exp(zmu + 0.5*zs^2); F = ev*zs
                s2h = wk.tile([C, NT], dt, tag="s2h")
                nc.scalar.activation(
                    out=s2h[:, :n], in_=zs_t[:, :n],
                    func=mybir.ActivationFunctionType.Square, scale=HALF_SQ,
                )
                w = wk.tile([C, NT], dt, tag="w")
                nc.vector.tensor_add(w[:, :n], s2h[:, :n], zmu_t[:, :n])
                ev = wk.tile([C, NT], dt, tag="ev")
                nc.scalar.activation(
                    out=ev[:, :n], in_=w[:, :n],
                    func=mybir.ActivationFunctionType.Exp,
                )
                F = wk.tile([C, NT], dt, tag="F")
                nc.vector.tensor_mul(F[:, :n], ev[:, :n], zs_t[:, :n])

                ssum_ps = ps_s.tile([S, NT], dt)
                nc.tensor.matmul(
                    ssum_ps[:, :n], cpt_t, ev[:, :n], start=True, stop=True
                )
                s_sb = wk.tile([S, NT], dt, tag="s_sb")
                nc.vector.tensor_copy(s_sb[:, :n], ssum_ps[:, :n])
                nc.sync.dma_start(out=bigS[rows, :n], in_=s_sb[:, :n])

                var_ps = ps_v.tile([S, NT], dt)
                for d in range(NDBL):
                    y_ps = ps_y.tile([128, 2, NT], dt)
                    for j in range(2):
                        p = 2 * d + j
                        nc.tensor.matmul(
                            y_ps[:, j, :n], lhsp_t[:, p, :], F[:, :n],
                            start=True, stop=True,
                        )
                    x = xp.tile([128, 2, NT], dt)
                    if d < ACT_DOUBLES:
                        nc.scalar.activation(
                            out=x, in_=y_ps,
                            func=mybir.ActivationFunctionType.Square,
                        )
                    else:
                        y_sb = ye.tile([128, 2, NT], dt)
                        nc.vector.tensor_copy(y_sb, y_ps)
                        nc.vector.tensor_mul(x, y_sb, y_sb)
                    for j in range(2):
                        p = 2 * d + j
                        nc.tensor.matmul(
                            var_ps[:, :n], ones_t[:, p, :], x[:, j, :n],
                            start=(p == 0), stop=(p == NPAIR - 1),
                        )
                v_sb = wk.tile([S, NT], dt, tag="v_sb")
                nc.vector.tensor_copy(v_sb[:, :n], var_ps[:, :n])
                nc.sync.dma_start(out=bigV[rows, :n], in_=v_sb[:, :n])

            # tail on (80, NT): c = lnv - 2 lnS; sigma = exp(c/2);
            # v = exp(c); mu = lnS - v/2
            lnS = big.tile([SP, NT], dt)
            nc.scalar.activation(
                out=lnS, in_=bigS, func=mybir.ActivationFunctionType.Ln
            )
            lnv = big.tile([SP, NT], dt)
            nc.scalar.activation(
                out=lnv, in_=bigV, func=mybir.ActivationFunctionType.Ln
            )
            c2 = big.tile([SP, NT], dt)
            nc.vector.tensor_scalar_mul(c2, lnS, -2.0)
            c3 = big.tile([SP, NT], dt)
            nc.vector.tensor_add(c3, c2, lnv)
            sg_t = big.tile([SP, NT], dt)
            nc.scalar.activation(
                out=sg_t, in_=c3, func=mybir.ActivationFunctionType.Exp,
                scale=0.5,
            )
            v_t = big.tile([SP, NT], dt)
            nc.scalar.activation(
                out=v_t, in_=c3, func=mybir.ActivationFunctionType.Exp
            )
            h_t = big.tile([SP, NT], dt)
            nc.vector.tensor_scalar_mul(h_t, v_t, -0.5)
            mu_t = big.tile([SP, NT], dt)
            nc.vector.tensor_add(mu_t, lnS, h_t)

            for i, (g0, n) in enumerate(CHUNKS):
                sl = slice(g0, g0 + n)
                rows = slice(S * i, S * (i + 1))
                nc.sync.dma_start(out=mu_d[:, sl], in_=mu_t[rows, :n])
                nc.sync.dma_start(out=sg_d[:, sl], in_=sg_t[rows, :n])

    nc.compile()
    return nc


def _get_nc():
    if "nc" not in _cache:
        _cache["nc"] = _build()
    return _cache["nc"]


def _host_prep(corr, cell_prob):
    R = np.linalg.cholesky(corr.astype(np.float64)).astype(np.float32)
    cp = cell_prob.astype(np.float32)
    lhsp = np.zeros((C, NPAIR, 128), np.float32)
    for p in range(NPAIR):
        lhsp[:, p, 0:64] = cp[2 * p][:, None] * R
        lhsp[:, p, 64:128] = cp[2 * p + 1][:, None] * R
    cpt = np.ascontiguousarray(cp.T)
    ones = np.zeros((128, NPAIR, S), np.float32)
    for p in range(NPAIR):
        ones[0:64, p, 2 * p] = 1.0
        ones[64:128, p, 2 * p + 1] = 1.0
    return lhsp, cpt, ones


def kernel(Z_mu, Z_sigma, corr, cell_prob, _trace=False):
    Z_mu = np.asarray(Z_mu, np.float32)
    Z_sigma = np.asarray(Z_sigma, np.float32)
    lhsp, cpt, ones = _host_prep(np.asarray(corr), np.asarray(cell_prob))

    in_maps = []
    for i in range(NCORES):
        sl = slice(i * GLOC, (i + 1) * GLOC)
        in_maps.append({
            "zmu": np.ascontiguousarray(Z_mu[:, sl]),
            "zs": np.ascontiguousarray(Z_sigma[:, sl]),
            "lhsp": lhsp,
            "cpt": cpt,
            "ones": ones,
        })

    nc = _get_nc()
    res = run_bass_kernel_spmd(
        nc, in_maps, core_ids=list(range(NCORES)), trace=_trace
    )
    mu = np.concatenate([r["mu"] for r in res.results], axis=1)
    sigma = np.concatenate([r["sigma"] for r in res.results], axis=1)
    if _trace:
        _cache["last_results"] = res
    return (mu, sigma)


# revision 20
# speedup vs baseline: 1.5218x; 1.5218x over previous
"""LogNorm moment-matching kernel for Trainium2, sharded over 8 NeuronCores.

Math: given Z_mu, Z_sigma (C=64, G=20000), corr (C,C), cell_prob (S=16, C):
  ev    = exp(Z_mu + 0.5*Z_sigma^2)
  Ssum  = cell_prob @ ev                                  (S, G)
  var_n[s,g] = sum_{c,d} A[s,c,g] corr[c,d] A[s,d,g],  A = cell_prob[s,c]*F[c,g],
               F = ev * Z_sigma
With corr = R R^T (Cholesky):  var_n[s] = || R^T diag(cp[s]) F ||^2
  -> per state: one 64x64 matmul (2 states packed per 128-col lhsT), squares of
     the result, then a partition-sum done as an accumulating matmul with a 0/1
     selector lhsT.
  var = var_n / Ssum^2; mu = ln(Ssum) - var/2; sigma = sqrt(var)
  done via the exp/ln table set only: c = ln(var_n) - 2 ln(Ssum),
  sigma = exp(0.5 c), var = exp(c).

Sharding: genes split 20000 -> 8 x 2500; corr/cell_prob-derived constants
replicated. All G-sized compute on device.

Engine split per gene chunk (NT=512): PE: 8 Y-matmuls + 8 reduce + 1 Ssum.
ACT: square-prep, exp, 2 of 4 double-pair squares (PSUM->SBUF). DVE: other 2
doubles (after DMA PSUM->SBUF evict). GPSIMD: the two SBUF elementwise muls.
Finals on a (80, 512) restacked layout to use full partitions.
"""

import sys

sys.path.insert(0, "/opt/trn_rl_repo")

import numpy as np

import concourse.bass as bass
import concourse.tile as tile
from concourse import bacc, mybir
from concourse.bass_utils import run_bass_kernel_spmd

S, C, G = 16, 64, 20000
NCORES = 8
GLOC = G // NCORES          # 2500 genes per core
NT = 512                    # gene tile = one PSUM bank of f32
NPAIR = S // 2
NDBL = NPAIR // 2           # 4 double-pair PSUM tiles per chunk
ACT_DOUBLES = 3             # doubles squared directly by ACT; rest DVE copy+mul

_cache = {}


def _chunks():
    out = []
    g = 0
    while g < GLOC:
        n = min(NT, GLOC - g)
        out.append((g, n))
        g += n
    return out


CHUNKS = _chunks()
NCHUNK = len(CHUNKS)


def _build():
    nc = bacc.Bacc("TRN2", target_bir_lowering=False)
    dt = mybir.dt.float32
    dtr = mybir.dt.float32r

    zmu_d = nc.dram_tensor("zmu", [C, GLOC], dt, kind="ExternalInput")
    zs_d = nc.dram_tensor("zs", [C, GLOC], dt, kind="ExternalInput")
    lhsp_d = nc.dram_tensor("lhsp", [C, NPAIR, 128], dtr, kind="ExternalInput")
    cpt_d = nc.dram_tensor("cpt", [C, S], dt, kind="ExternalInput")
    ones_d = nc.dram_tensor("ones", [128, NPAIR, S], dtr, kind="ExternalInput")
    mu_d = nc.dram_tensor("mu", [S, GLOC], dt, kind="ExternalOutput")
    sg_d = nc.dram_tensor("sigma", [S, GLOC], dt, kind="ExternalOutput")

    HALF_SQ = float(np.sqrt(0.5))
    SP = S * NCHUNK  # 80 partitions for the restacked tail

    with tile.TileContext(nc) as tc:
        with (
            tc.tile_pool(name="consts", bufs=1) as consts,
            tc.tile_pool(name="inp", bufs=3) as inp,
            tc.tile_pool(name="wk", bufs=3) as wk,
            tc.tile_pool(name="xp", bufs=4) as xp,
            tc.tile_pool(name="ye", bufs=3) as ye,
            tc.tile_pool(name="big", bufs=1) as big,
            tc.tile_pool(name="ps_y", bufs=3, space="PSUM") as ps_y,
            tc.tile_pool(name="ps_s", bufs=1, space="PSUM") as ps_s,
            tc.tile_pool(name="ps_v", bufs=1, space="PSUM") as ps_v,
        ):
            lhsp_t = consts.tile([C, NPAIR, 128], dtr)
            nc.sync.dma_start(out=lhsp_t, in_=lhsp_d[:, :, :])
            cpt_t = consts.tile([C, S], dt)
            nc.sync.dma_start(out=cpt_t, in_=cpt_d[:, :])
            ones_t = consts.tile([128, NPAIR, S], dtr)
            nc.sync.dma_start(out=ones_t, in_=ones_d[:, :, :])

            bigS = big.tile([SP, NT], dt)
            bigV = big.tile([SP, NT], dt)

            for i, (g0, n) in enumerate(CHUNKS):
                sl = slice(g0, g0 + n)
                rows = slice(S * i, S * (i + 1))
                zmu_t = inp.tile([C, NT], dt, tag="zmu")
                nc.gpsimd.dma_start(out=zmu_t[:, :n], in_=zmu_d[:, sl])
                zs_t = inp.tile([C, NT], dt, tag="zs")
                nc.gpsimd.dma_start(out=zs_t[:, :n], in_=zs_d[:, sl])

                # ev = exp(zmu + 0.5*zs^2); F = ev*zs
                s2h = wk.tile([C, NT], dt, tag="s2h")
                nc.scalar.activation(
                    out=s2h[:, :n], in_=zs_t[:, :n],
                    func=mybir.ActivationFunctionType.Square, scale=HALF_SQ,
                )
                w = wk.tile([C, NT], dt, tag="w")
                nc.vector.tensor_add(w[:, :n], s2h[:, :n], zmu_t[:, :n])
                ev = wk.tile([C, NT], dt, tag="ev")
                nc.scalar.activation(
                    out=ev[:, :n], in_=w[:, :n],
                    func=mybir.ActivationFunctionType.Exp,
                )
                F = wk.tile([C, NT], dtr, tag="F")
                nc.vector.tensor_mul(F[:, :n], ev[:, :n], zs_t[:, :n])

                ssum_ps = ps_s.tile([S, NT], dt)
                nc.tensor.matmul(
                    ssum_ps[:, :n], cpt_t, ev[:, :n], start=True, stop=True
                )
                s_sb = wk.tile([S, NT], dt, tag="s_sb")
                nc.vector.tensor_copy(s_sb[:, :n], ssum_ps[:, :n])
                nc.sync.dma_start(out=bigS[rows, :n], in_=s_sb[:, :n])

                var_ps = ps_v.tile([S, NT], dt)
                for d in range(NDBL):
                    y_ps = ps_y.tile([128, 2, NT], dt)
                    for j in range(2):
                        p = 2 * d + j
                        nc.tensor.matmul(
                            y_ps[:, j, :n], lhsp_t[:, p, :],
                            F[:, :n],
                            start=True, stop=True,
                        )
                    x = xp.tile([128, 2, NT], dtr)
                    if d < ACT_DOUBLES:
                        nc.scalar.activation(
                            out=x, in_=y_ps,
                            func=mybir.ActivationFunctionType.Square,
                        )
                    else:
                        y_sb = ye.tile([128, 2, NT], dt)
                        nc.vector.tensor_copy(y_sb, y_ps)
                        nc.vector.tensor_mul(x, y_sb, y_sb)
                    for j in range(2):
                        p = 2 * d + j
                        nc.tensor.matmul(
                            var_ps[:, :n], ones_t[:, p, :],
                            x[:, j, :n],
                            start=(p == 0), stop=(p == NPAIR - 1),
                        )
                v_sb = wk.tile([S, NT], dt, tag="v_sb")
                nc.vector.tensor_copy(v_sb[:, :n], var_ps[:, :n])
                nc.sync.dma_start(out=bigV[rows, :n], in_=v_sb[:, :n])

            # tail on (80, NT): c = lnv - 2 lnS; sigma = exp(c/2);
            # v = exp(c); mu = lnS - v/2
            lnS = big.tile([SP, NT], dt)
            nc.scalar.activation(
                out=lnS, in_=bigS, func=mybir.ActivationFunctionType.Ln
            )
            lnv = big.tile([SP, NT], dt)
            nc.scalar.activation(
                out=lnv, in_=bigV, func=mybir.ActivationFunctionType.Ln
            )
            c2 = big.tile([SP, NT], dt)
            nc.vector.tensor_scalar_mul(c2, lnS, -2.0)
            c3 = big.tile([SP, NT], dt)
            nc.vector.tensor_add(c3, c2, lnv)
            sg_t = big.tile([SP, NT], dt)
            nc.scalar.activation(
                out=sg_t, in_=c3, func=mybir.ActivationFunctionType.Exp,
                scale=0.5,
            )
            v_t = big.tile([SP, NT], dt)
            nc.scalar.activation(
                out=v_t, in_=c3, func=mybir.ActivationFunctionType.Exp
            )
            h_t = big.tile([SP, NT], dt)
            nc.vector.tensor_scalar_mul(h_t, v_t, -0.5)
            mu_t = big.tile([SP, NT], dt)
            nc.vector.tensor_add(mu_t, lnS, h_t)

            for i, (g0, n) in enumerate(CHUNKS):
                sl = slice(g0, g0 + n)
                rows = slice(S * i, S * (i + 1))
                nc.sync.dma_start(out=mu_d[:, sl], in_=mu_t[rows, :n])
                nc.sync.dma_start(out=sg_d[:, sl], in_=sg_t[rows, :n])

    nc.compile()
    return nc


def _get_nc():
    if "nc" not in _cache:
        _cache["nc"] = _build()
    return _cache["nc"]


def _host_prep(corr, cell_prob):
    R = np.linalg.cholesky(corr.astype(np.float64)).astype(np.float32)
    cp = cell_prob.astype(np.float32)
    lhsp = np.zeros((C, NPAIR, 128), np.float32)
    for p in range(NPAIR):
        lhsp[:, p, 0:64] = cp[2 * p][:, None] * R
        lhsp[:, p, 64:128] = cp[2 * p + 1][:, None] * R
    cpt = np.ascontiguousarray(cp.T)
    ones = np.zeros((128, NPAIR, S), np.float32)
    for p in range(NPAIR):
        ones[0:64, p, 2 * p] = 1.0
        ones[64:128, p, 2 * p + 1] = 1.0
    return lhsp, cpt, ones


def kernel(Z_mu, Z_sigma, corr, cell_prob, _trace=False):
    Z_mu = np.asarray(Z_mu, np.float32)
    Z_sigma = np.asarray(Z_sigma, np.float32)
    lhsp, cpt, ones = _host_prep(np.asarray(corr), np.asarray(cell_prob))

    in_maps = []
    for i in range(NCORES):
        sl = slice(i * GLOC, (i + 1) * GLOC)
        in_maps.append({
            "zmu": np.ascontiguousarray(Z_mu[:, sl]),
            "zs": np.ascontiguousarray(Z_sigma[:, sl]),
            "lhsp": lhsp,
            "cpt": cpt,
            "ones": ones,
        })

    nc = _get_nc()
    res = run_bass_kernel_spmd(
        nc, in_maps, core_ids=list(range(NCORES)), trace=_trace
    )
    mu = np.concatenate([r["mu"] for r in res.results], axis=1)
    sigma = np.concatenate([r["sigma"] for r in res.results], axis=1)
    if _trace:
        _cache["last_results"] = res
    return (mu, sigma)
